# revision 1
# baseline (speedup 1.0000x reference)
"""CrystalGNN (GCNConv -> relu -> mean-pool -> FC -> log_softmax) on 8
Trainium2 NeuronCores.

Strategy (graph/data parallel, per sharding hint):
- 256 graphs -> 8 cores x 32 graphs. batch_idx is sorted, so each core owns a
  contiguous node range and every edge's *target* (col) lives on exactly one
  core. Edges are sharded by target.
- Each core computes the full normalized feature table
  h'[n] = dinv[n] * (x[n] @ W1) (redundant across cores; cheap dense matmul)
  into DRAM as bf16 rows padded to 256B, split into 4 chunks of 25088 rows so
  dma_gather's int16 indices can address any row.
- Message passing: per target-window of 128 local nodes, gather all source
  rows with dma_gather (one 256B descriptor per edge incl. self-loop), then
  segment-sum with one-hot T matmuls on the tensor engine accumulating in
  PSUM. Window results get dinv[col] scaling + b1 + relu.
- Mean-pool + bias folded into a matmul against a host-built B matrix
  (B[n,g] = 1[batch[n]==g]/cnt[g]); final FC + log_softmax on device.

Self-contained: only needs numpy/ml_dtypes + the concourse stack at
/opt/trn_rl_repo (or already on sys.path).
"""
import sys

for _p in ("/opt/trn_rl_repo",):
    if _p not in sys.path:
        sys.path.append(_p)

import numpy as np
import ml_dtypes

import concourse.bass as bass
import concourse.bacc as bacc
import concourse.mybir as mybir
import concourse.tile as tile
from concourse import bass_utils

P = 128
NCORES = 8
NGRAPH = 256
GPC = NGRAPH // NCORES        # graphs per core
N = 100000                    # nodes
FIN = 128                     # input features
H = 64                        # hidden
CHUNK = 25088                 # h' table chunk rows (196*128, int16-addressable)
NCHUNK = 4
NTAB = CHUNK * NCHUNK         # 100352 padded table rows
SW = 4                        # windows per superwindow (PSUM banks)
HSTRIP = 14                   # h-phase blocks per DMA strip (196 = 14*14)
CALLCAP = 8                   # max gather blocks (x128 idx) per dma_gather call

bf16 = ml_dtypes.bfloat16


# ----------------------------------------------------------------- schedule
def build_schedule(x, edge_index, batch_idx, W1, b1, Wfc, bfc):
    """Host-side preprocessing: sharding, slot schedule, index/one-hot data."""
    x = np.asarray(x)
    edge_index = np.asarray(edge_index).astype(np.int64)
    batch = np.asarray(batch_idx).astype(np.int64)
    W1 = np.asarray(W1, dtype=np.float32)
    b1 = np.asarray(b1, dtype=np.float32)
    Wfc = np.asarray(Wfc, dtype=np.float32)
    bfc = np.asarray(bfc, dtype=np.float32)

    row, col = edge_index[0], edge_index[1]
    deg = np.bincount(col, minlength=N).astype(np.float32) + 1.0
    dinv = (1.0 / np.sqrt(deg)).astype(np.float32)

    gcnt = np.bincount(batch, minlength=NGRAPH).astype(np.float32)
    assert (gcnt > 0).all(), "empty graphs unsupported"
    inv_cnt = 1.0 / gcnt

    starts = np.searchsorted(batch, np.arange(NCORES) * GPC, side="left")
    ends = np.searchsorted(batch, (np.arange(NCORES) + 1) * GPC, side="left")
    nk = ends - starts
    NWIN = int(np.ceil(nk.max() / P))
    NSW = (NWIN + SW - 1) // SW

    ecore = np.searchsorted(ends, col, side="right")  # core of each edge (by target)

    # per-core edge arrays (target-sharded) with self-loops appended
    core_src, core_w, core_c, core_cl = [], [], [], []
    for k in range(NCORES):
        m = ecore == k
        r_k = row[m]
        c_k = col[m]
        nloc = np.arange(starts[k], ends[k], dtype=np.int64)
        src = np.concatenate([r_k, nloc])
        tgt = np.concatenate([c_k, nloc]) - starts[k]
        core_src.append(src)
        core_w.append(tgt >> 7)
        core_cl.append(tgt & 127)
        core_c.append(src // CHUNK)

    # shared block schedule: B[w, c] = ceil(max_k slots_k(w,c) / 128), >= 1
    Btab = np.ones((NWIN, NCHUNK), dtype=np.int64)
    for k in range(NCORES):
        key = core_w[k] * NCHUNK + core_c[k]
        cnt = np.bincount(key, minlength=NWIN * NCHUNK).reshape(NWIN, NCHUNK)
        Btab = np.maximum(Btab, (cnt + P - 1) // P)

    # emission order: for sw, for chunk, for w in sw, blocks
    blk_w, blk_c = [], []
    call_cb = []          # blocks per call (sw, c)
    blk_base = np.zeros((NWIN, NCHUNK), dtype=np.int64)
    K = 0
    for s in range(NSW):
        ws = range(s * SW, min((s + 1) * SW, NWIN))
        for c in range(NCHUNK):
            cb = 0
            for w in ws:
                blk_base[w, c] = K
                b = int(Btab[w, c])
                blk_w += [w] * b
                blk_c += [c] * b
                K += b
                cb += b
            call_cb.append(cb)
    TOTBLK = K
    TOTSLOT = TOTBLK * P

    # per-core slot data
    idx16 = np.zeros((NCORES, P, TOTSLOT // 16), dtype=np.int16)
    colv = np.full((NCORES, P, TOTBLK), -1.0, dtype=np.float32)
    for k in range(NCORES):
        key = core_w[k] * NCHUNK + core_c[k]
        order = np.argsort(key, kind="stable")
        skey = key[order]
        ssrc = core_src[k][order]
        scl = core_cl[k][order]
        seg_start = np.searchsorted(skey, np.arange(NWIN * NCHUNK), side="left")
        seg_end = np.searchsorted(skey, np.arange(NWIN * NCHUNK), side="right")
        gidx = np.zeros(TOTSLOT, dtype=np.int16)
        for w in range(NWIN):
            for c in range(NCHUNK):
                a, b = seg_start[w * NCHUNK + c], seg_end[w * NCHUNK + c]
                n = b - a
                if n == 0:
                    continue
                base = blk_base[w, c] * P
                pos = base + np.arange(n)
                gidx[pos] = (ssrc[a:b] - c * CHUNK).astype(np.int16)
                colv[k, pos & 127, pos >> 7] = scl[a:b].astype(np.float32)
        # wrap per call: within call, wrapped[q, s] = idx[s*16+q]
        off = 0
        off16 = 0
        for cb in call_cb:
            nslot = cb * P
            wr = gidx[off:off + nslot].reshape(-1, 16).T  # [16, nslot//16]
            idx16[k, :, off16:off16 + nslot // 16] = np.tile(wr, (8, 1))
            off += nslot
            off16 += nslot // 16

    # B pooling matrix and dinv per window
    Bmat = np.zeros((NCORES, P, NWIN * GPC), dtype=bf16)
    dinv_win = np.zeros((NCORES, P, NWIN), dtype=np.float32)
    for k in range(NCORES):
        nn = int(nk[k])
        nodes = np.arange(starts[k], ends[k])
        g = batch[nodes] - k * GPC
        w = np.arange(nn) >> 7
        p = np.arange(nn) & 127
        Bm = np.zeros((P, NWIN, GPC), dtype=np.float32)
        Bm[p, w, g] = inv_cnt[batch[nodes]]
        Bmat[k] = Bm.reshape(P, NWIN * GPC).astype(bf16)
        dv = np.zeros((P, NWIN), dtype=np.float32)
        dv[p, w] = dinv[nodes]
        dinv_win[k] = dv

    # shared tensors
    xT = np.zeros((FIN, NTAB), dtype=bf16)
    xT[:, :N] = np.asarray(x, dtype=np.float32).T.astype(bf16)
    dinv_pad = np.zeros(NTAB, dtype=np.float32)
    dinv_pad[:N] = dinv
    dinv_blk = np.ascontiguousarray(dinv_pad.reshape(NTAB // P, P).T)  # [p, b] = dinv[128*b+p]
    b1b = np.broadcast_to(b1, (P, H)).astype(np.float32).copy()
    wfce = np.concatenate([Wfc, bfc[None, :]], axis=0).astype(np.float32)  # [65, 2]
    iota = np.broadcast_to(np.arange(P, dtype=np.float32), (P, P)).copy()
    ident = np.eye(P, dtype=np.float32)

    return dict(
        NWIN=NWIN, NSW=NSW, Btab=Btab, call_cb=call_cb, TOTBLK=TOTBLK,
        idx16=idx16, colv=colv, Bmat=Bmat, dinv_win=dinv_win,
        xT=xT, dinv_blk=dinv_blk, b1b=b1b, wfce=wfce, iota=iota, ident=ident,
        W1=W1.astype(bf16),
    )


# ------------------------------------------------------------------ kernel IR
def build_nc(sched, num_devices=NCORES):
    NWIN, NSW = sched["NWIN"], sched["NSW"]
    Btab, call_cb, TOTBLK = sched["Btab"], sched["call_cb"], sched["TOTBLK"]
    f32, bft, i16 = mybir.dt.float32, mybir.dt.bfloat16, mybir.dt.int16

    nc = bacc.Bacc("TRN2", target_bir_lowering=False, debug=False,
                   num_devices=num_devices)
    d_xT = nc.dram_tensor("xT", [FIN, NTAB], bft, kind="ExternalInput")
    d_W1 = nc.dram_tensor("W1", [FIN, H], bft, kind="ExternalInput")
    d_dblk = nc.dram_tensor("dinv_blk", [P, NTAB // P], f32, kind="ExternalInput")
    d_idx = nc.dram_tensor("idx16", [P, TOTBLK * 8], i16, kind="ExternalInput")
    d_colv = nc.dram_tensor("colv", [P, TOTBLK], f32, kind="ExternalInput")
    d_B = nc.dram_tensor("Bmat", [P, NWIN * GPC], bft, kind="ExternalInput")
    d_dwin = nc.dram_tensor("dinv_win", [P, NWIN], f32, kind="ExternalInput")
    d_b1b = nc.dram_tensor("b1b", [P, H], f32, kind="ExternalInput")
    d_wfce = nc.dram_tensor("wfce", [H + 1, 2], f32, kind="ExternalInput")
    d_iota = nc.dram_tensor("iota", [P, P], f32, kind="ExternalInput")
    d_ident = nc.dram_tensor("ident", [P, P], f32, kind="ExternalInput")
    d_out = nc.dram_tensor("outd", [GPC, 2], f32, kind="ExternalOutput")

    NBLK_H = CHUNK // P  # 196 h-blocks per chunk

    with tile.TileContext(nc) as tc:
        with tc.tile_pool(name="const", bufs=1) as cp, \
             tc.tile_pool(name="hio", bufs=3) as hio, \
             tc.tile_pool(name="gio", bufs=3) as gio, \
             tc.tile_pool(name="tp", bufs=4) as tpool, \
             tc.tile_pool(name="wio", bufs=3) as wio, \
             tc.tile_pool(name="hps", bufs=2, space="PSUM") as hps, \
             tc.tile_pool(name="aggps", bufs=SW, space="PSUM") as aggps, \
             tc.tile_pool(name="poolps", bufs=1, space="PSUM") as poolps, \
             tc.tile_pool(name="dram", bufs=1, space="DRAM") as dp:

            # constants
            w1_t = cp.tile([FIN, H], bft, tag="w1")
            nc.sync.dma_start(out=w1_t[:], in_=d_W1[:])
            dblk_t = cp.tile([P, NTAB // P], f32, tag="dblk")
            nc.sync.dma_start(out=dblk_t[:], in_=d_dblk[:])
            iota_t = cp.tile([P, P], f32, tag="iota")
            nc.sync.dma_start(out=iota_t[:], in_=d_iota[:])
            ident_t = cp.tile([P, P], f32, tag="ident")
            nc.sync.dma_start(out=ident_t[:], in_=d_ident[:])
            b1b_t = cp.tile([P, H], f32, tag="b1b")
            nc.sync.dma_start(out=b1b_t[:], in_=d_b1b[:])
            wfce_t = cp.tile([H + 1, 2], f32, tag="wfce")
            nc.sync.dma_start(out=wfce_t[:], in_=d_wfce[:])
            idx_t = cp.tile([P, TOTBLK * 8], i16, tag="idx")
            nc.sync.dma_start(out=idx_t[:], in_=d_idx[:])
            colv_t = cp.tile([P, TOTBLK], f32, tag="colv")
            nc.sync.dma_start(out=colv_t[:], in_=d_colv[:])
            bmat_t = cp.tile([P, NWIN * GPC], bft, tag="bmat")
            nc.sync.dma_start(out=bmat_t[:], in_=d_B[:])
            dwin_t = cp.tile([P, NWIN], f32, tag="dwin")
            nc.sync.dma_start(out=dwin_t[:], in_=d_dwin[:])
            ones_t = cp.tile([P, 1], bft, tag="ones")
            nc.vector.memset(ones_t[:], 1.0)

            # h' chunk tables in DRAM (bf16 rows padded to 256B: [r, 128])
            hbuf = [dp.tile([CHUNK, P], bft, tag=f"hbuf{c}", name=f"hbuf{c}")
                    for c in range(NCHUNK)]

            # ---- phase 1: h' = dinv * (x @ W1), streamed per chunk ----
            for c in range(NCHUNK):
                for s in range(NBLK_H // HSTRIP):
                    b0 = c * NBLK_H + s * HSTRIP  # global block
                    xs = hio.tile([P, HSTRIP * P], bft, tag="xs")
                    nc.sync.dma_start(
                        out=xs[:], in_=d_xT[:, b0 * P:(b0 + HSTRIP) * P])
                    hst = hio.tile([P, HSTRIP, H], bft, tag="hst")
                    for j in range(HSTRIP):
                        hp = hps.tile([P, H], f32, tag="hp")
                        nc.tensor.matmul(
                            out=hp[:], lhsT=xs[:, j * P:(j + 1) * P],
                            rhs=w1_t[:], start=True, stop=True)
                        nc.vector.tensor_scalar(
                            out=hst[:, j, :], in0=hp[:],
                            scalar1=dblk_t[:, b0 + j:b0 + j + 1], scalar2=None,
                            op0=mybir.AluOpType.mult)
                    # store rows [s*HSTRIP*P, ...) of chunk c (cols 0:64)
                    dst = hbuf[c][s * HSTRIP * P:(s + 1) * HSTRIP * P, 0:H]
                    nc.sync.dma_start(
                        out=dst.rearrange("(j p) h -> p j h", p=P), in_=hst[:])

            # ---- phase 2: gather + segment-sum + pool ----
            pool_ps = poolps.tile([H + 1, GPC], f32, tag="pool")
            blk = 0      # global block counter
            off16 = 0    # idx16 column offset
            nwin_done = 0
            for s in range(NSW):
                ws = list(range(s * SW, min((s + 1) * SW, NWIN)))
                agg = {w: aggps.tile([P, H], f32, tag="agg", name=f"agg{w}")
                       for w in ws}
                for c in range(NCHUNK):
                    # ordered blocks of this (sw, chunk) call group
                    blist = []
                    for w in ws:
                        nb = int(Btab[w, c])
                        for b in range(nb):
                            blist.append((w, c == 0 and b == 0,
                                          c == NCHUNK - 1 and b == nb - 1))
                    # gather in sub-calls of <= CALLCAP blocks (SWDGE ring cap)
                    for g0 in range(0, len(blist), CALLCAP):
                        grp = blist[g0:g0 + CALLCAP]
                        cb = len(grp)
                        msg = gio.tile([P, CALLCAP, P], bft, tag="msg")
                        nc.gpsimd.dma_gather(
                            out_ap=msg[:, 0:cb, :], in_ap=hbuf[c][:],
                            idxs_ap=idx_t[:, off16:off16 + cb * 8],
                            num_idxs=cb * P, num_idxs_reg=cb * P, elem_size=P)
                        off16 += cb * 8
                        for bi, (w, first, last) in enumerate(grp):
                            T = tpool.tile([P, P], bft, tag="T")
                            nc.vector.tensor_scalar(
                                out=T[:], in0=iota_t[:],
                                scalar1=colv_t[:, blk:blk + 1], scalar2=None,
                                op0=mybir.AluOpType.is_equal)
                            nc.tensor.matmul(
                                out=agg[w][:], lhsT=T[:],
                                rhs=msg[:, bi, 0:H],
                                start=first, stop=last)
                            blk += 1
                # window epilogue: scale, bias, relu, pool
                for w in ws:
                    sc = wio.tile([P, H], f32, tag="sc")
                    nc.vector.tensor_scalar(
                        out=sc[:], in0=agg[w][:], scalar1=dwin_t[:, w:w + 1],
                        scalar2=None, op0=mybir.AluOpType.mult)
                    sb = wio.tile([P, H], f32, tag="sb")
                    nc.vector.tensor_tensor(
                        out=sb[:], in0=sc[:], in1=b1b_t[:],
                        op=mybir.AluOpType.add)
                    rl = wio.tile([P, H], bft, tag="rl")
                    nc.scalar.activation(
                        out=rl[:], in_=sb[:],
                        func=mybir.ActivationFunctionType.Relu)
                    first = nwin_done == 0
                    last = nwin_done == NWIN - 1
                    nc.tensor.matmul(
                        out=pool_ps[0:H, :], lhsT=rl[:],
                        rhs=bmat_t[:, w * GPC:(w + 1) * GPC],
                        start=first, stop=last, skip_group_check=True)
                    nc.tensor.matmul(
                        out=pool_ps[H:H + 1, :], lhsT=ones_t[:],
                        rhs=bmat_t[:, w * GPC:(w + 1) * GPC],
                        start=first, stop=last, skip_group_check=True)
                    nwin_done += 1

            # ---- phase 3: FC + log_softmax ----
            plc = cp.tile([H + 1, GPC], f32, tag="plc")
            nc.vector.tensor_copy(out=plc[:], in_=pool_ps[:])
            lg_ps = hps.tile([2, GPC], f32, tag="hp")
            nc.tensor.matmul(out=lg_ps[:], lhsT=wfce_t[:], rhs=plc[:],
                             start=True, stop=True)
            lgs = cp.tile([2, GPC], f32, tag="lgs")
            nc.vector.tensor_copy(out=lgs[:], in_=lg_ps[:])
            tr_ps = hps.tile([GPC, 2], f32, tag="hp")
            nc.tensor.transpose(out=tr_ps[:], in_=lgs[:], identity=ident_t[:2, :2])
            ls = cp.tile([GPC, 2], f32, tag="ls")
            nc.vector.tensor_copy(out=ls[:], in_=tr_ps[:])
            nm = cp.tile([GPC, 1], f32, tag="nm")
            nc.vector.tensor_reduce(out=nm[:], in_=ls[:],
                                    axis=mybir.AxisListType.X,
                                    op=mybir.AluOpType.max, negate=True)
            ex = cp.tile([GPC, 2], f32, tag="ex")
            nc.scalar.activation(out=ex[:], in_=ls[:],
                                 func=mybir.ActivationFunctionType.Exp,
                                 bias=nm[:, 0:1])
            ssum = cp.tile([GPC, 1], f32, tag="ssum")
            nc.vector.tensor_reduce(out=ssum[:], in_=ex[:],
                                    axis=mybir.AxisListType.X,
                                    op=mybir.AluOpType.add)
            lse = cp.tile([GPC, 1], f32, tag="lse")
            nc.scalar.activation(out=lse[:], in_=ssum[:],
                                 func=mybir.ActivationFunctionType.Ln)
            fin = cp.tile([GPC, 2], f32, tag="fin")
            nc.vector.tensor_scalar(
                out=fin[:], in0=ls[:], scalar1=nm[:, 0:1], scalar2=lse[:, 0:1],
                op0=mybir.AluOpType.add, op1=mybir.AluOpType.subtract)
            nc.sync.dma_start(out=d_out[:], in_=fin[:])

    nc.compile()
    return nc


def make_in_maps(sched):
    maps = []
    for k in range(NCORES):
        maps.append({
            "xT": sched["xT"], "W1": sched["W1"],
            "dinv_blk": sched["dinv_blk"],
            "idx16": sched["idx16"][k], "colv": sched["colv"][k],
            "Bmat": sched["Bmat"][k], "dinv_win": sched["dinv_win"][k],
            "b1b": sched["b1b"], "wfce": sched["wfce"],
            "iota": sched["iota"], "ident": sched["ident"],
        })
    return maps


def kernel(**inputs) -> np.ndarray:
    sched = build_schedule(**inputs)
    nc = build_nc(sched)
    res = bass_utils.run_bass_kernel_spmd(
        nc, make_in_maps(sched), core_ids=list(range(NCORES)))
    out = np.concatenate([res.results[k]["outd"] for k in range(NCORES)], axis=0)
    return out.astype(np.float32)



# revision 4
# speedup vs baseline: 2.3312x; 2.3312x over previous
"""CrystalGNN (GCNConv -> relu -> mean-pool -> FC -> log_softmax) on 8
Trainium2 NeuronCores — v2 (pair-cell gather).

Bottleneck analysis of v1: SWDGE descriptor generation on GpSimd costs
~7.7 ns per gather index regardless of element size, so the per-edge
256B gather (~290k slots/core) pinned GpSimd at ~2.4 ms.

v2 halves the index count: the host pairs two same-window edges into one
512B "cell" (two adjacent 256B bf16 x-rows, pre-scaled by dinv[src]) in a
per-core reordered/duplicated node table (<= 2N rows, 3 chunks of 32768
cells for int16 indexing). One dma_gather index now feeds two edges. The
segment-sum stays on the tensor engine: per gathered block, per (window,
half) "piece", a one-hot T (built batched on DVE via stride-0 broadcast
is_equal) routes message halves into per-window PSUM accumulators in
x-space; each window then applies W1, dinv[tgt], bias, relu, and the
mean-pool matmul. Graph/data parallel across 8 cores per the sharding
hint (batch_idx is sorted, so each core owns 32 graphs and the edges
targeting them).

Self-contained: only needs numpy/ml_dtypes + the concourse stack at
/opt/trn_rl_repo (or already on sys.path).
"""
import sys

for _p in ("/opt/trn_rl_repo",):
    if _p not in sys.path:
        sys.path.append(_p)

import numpy as np
import ml_dtypes

import concourse.bass as bass
import concourse.bacc as bacc
import concourse.mybir as mybir
import concourse.tile as tile
from concourse import bass_utils

P = 128
NCORES = 8
NGRAPH = 256
GPC = NGRAPH // NCORES        # graphs per core
N = 100000                    # nodes
FIN = 128                     # input features
H = 64                        # hidden
SW = 4                        # windows per superwindow (PSUM agg tiles)
CALLCAP = 8                   # max gather blocks (x128 idx) per dma_gather call
CHUNK_CELLS = 32768           # cells addressable by int16 per chunk
NCHUNK = 3
NCELL_CAP = NCHUNK * CHUNK_CELLS   # per-core cell budget (table <= 2N rows)

bf16 = ml_dtypes.bfloat16


# ----------------------------------------------------------------- schedule
def _pair_core(src, w, t, ncell_cap):
    """Greedy pairing of one core's edges into cells.

    src: global source node per edge; w: target window; t: target slot in
    window (0..127). Returns (cells, per-edge cell/half arrays).
    cells: int64 [ncells, 2] node ids (row 2c = cells[c,0], 2c+1 = cells[c,1]).
    """
    E = len(src)
    order = np.lexsort((np.arange(E), w))        # stable sort by window
    src_o, w_o, t_o = src[order], w[order], t[order]

    # first occurrence of each node (in window-processed order) -> "new"
    po = np.argsort(src_o, kind="stable")
    first = np.ones(E, dtype=bool)
    same = src_o[po[1:]] == src_o[po[:-1]]
    first[po[1:][same]] = False

    cell_of = np.full(E, -1, dtype=np.int64)     # cell id per (sorted) edge
    half_of = np.zeros(E, dtype=np.int8)

    cells_a = []                                  # lo node of each cell
    cells_b = []                                  # hi node
    placed_cell = np.full(N, -1, dtype=np.int64)
    placed_half = np.zeros(N, dtype=np.int8)

    wbound = np.searchsorted(w_o, np.arange(w_o[-1] + 2))
    ncells = 0
    # pass 1: new-new pairs (and leftover new edge joins an old edge)
    pending_old = []                              # per-window old-edge idx lists
    for wi in range(len(wbound) - 1):
        a, b = wbound[wi], wbound[wi + 1]
        if a == b:
            pending_old.append(np.empty(0, dtype=np.int64))
            continue
        idxs = np.arange(a, b)
        newm = first[a:b]
        news = idxs[newm]
        olds = idxs[~newm]
        k = len(news) // 2
        if k:
            lo, hi = news[0:2 * k:2], news[1:2 * k:2]
            ids = ncells + np.arange(k)
            cell_of[lo] = ids; half_of[lo] = 0
            cell_of[hi] = ids; half_of[hi] = 1
            cells_a.append(src_o[lo]); cells_b.append(src_o[hi])
            placed_cell[src_o[lo]] = ids; placed_half[src_o[lo]] = 0
            placed_cell[src_o[hi]] = ids; placed_half[src_o[hi]] = 1
            ncells += k
        if len(news) % 2:
            u = news[-1]
            if len(olds):
                v = olds[0]
                olds = olds[1:]
                cell_of[u] = ncells; half_of[u] = 0
                cell_of[v] = ncells; half_of[v] = 1
                cells_a.append(src_o[u:u + 1]); cells_b.append(src_o[v:v + 1])
            else:
                cell_of[u] = ncells; half_of[u] = 0
                cells_a.append(src_o[u:u + 1]); cells_b.append(src_o[u:u + 1])
            placed_cell[src_o[u]] = ncells; placed_half[src_o[u]] = 0
            ncells += 1
        pending_old.append(olds)

    # pass 2: old-old dup pairs while budget lasts, else reuse singles
    for wi, olds in enumerate(pending_old):
        n = len(olds)
        if n == 0:
            continue
        budget = max(0, (ncell_cap - ncells))
        k = min(n // 2, budget)
        if k:
            lo, hi = olds[0:2 * k:2], olds[1:2 * k:2]
            ids = ncells + np.arange(k)
            cell_of[lo] = ids; half_of[lo] = 0
            cell_of[hi] = ids; half_of[hi] = 1
            cells_a.append(src_o[lo]); cells_b.append(src_o[hi])
            ncells += k
        rest = olds[2 * k:]
        if len(rest):
            cell_of[rest] = placed_cell[src_o[rest]]
            half_of[rest] = placed_half[src_o[rest]]

    assert (cell_of >= 0).all()
    assert ncells <= ncell_cap, ncells
    cells = np.stack([np.concatenate(cells_a), np.concatenate(cells_b)], axis=1)
    # undo the window sort
    inv = np.empty(E, dtype=np.int64)
    inv[order] = np.arange(E)
    return cells, cell_of[inv], half_of[inv]


def build_schedule(x, edge_index, batch_idx, W1, b1, Wfc, bfc):
    x = np.asarray(x, dtype=np.float32)
    edge_index = np.asarray(edge_index).astype(np.int64)
    batch = np.asarray(batch_idx).astype(np.int64)
    W1 = np.asarray(W1, dtype=np.float32)
    b1 = np.asarray(b1, dtype=np.float32)
    Wfc = np.asarray(Wfc, dtype=np.float32)
    bfc = np.asarray(bfc, dtype=np.float32)

    row, col = edge_index[0], edge_index[1]
    deg = np.bincount(col, minlength=N).astype(np.float32) + 1.0
    dinv = (1.0 / np.sqrt(deg)).astype(np.float32)

    gcnt = np.bincount(batch, minlength=NGRAPH).astype(np.float32)
    assert (gcnt > 0).all(), "empty graphs unsupported"
    inv_cnt = 1.0 / gcnt

    starts = np.searchsorted(batch, np.arange(NCORES) * GPC, side="left")
    ends = np.searchsorted(batch, (np.arange(NCORES) + 1) * GPC, side="left")
    nk = ends - starts
    NWIN = int(np.ceil(nk.max() / P))
    NSW = (NWIN + SW - 1) // SW

    xs = (x * dinv[:, None]).astype(bf16)         # dinv[src]-scaled rows

    ecore = np.searchsorted(ends, col, side="right")

    # per-core pairing
    core = []
    for k in range(NCORES):
        m = ecore == k
        nloc = np.arange(starts[k], ends[k], dtype=np.int64)
        esrc = np.concatenate([row[m], nloc])
        etl = np.concatenate([col[m] - starts[k], nloc - starts[k]])
        ew = etl >> 7
        et = etl & 127
        cells, cell_of, half_of = _pair_core(esrc, ew, et, NCELL_CAP)
        core.append(dict(src=esrc, w=ew, t=et, cells=cells,
                         cell=cell_of, half=half_of))

    # ---- shared slot layout -------------------------------------------
    # per (w, chunk): slots; within a (w,c), per cell, lo/hi demand lists.
    # A slot = one gathered cell instance; its lo half serves <=1 edge, hi
    # half <=1 edge (same window). Per-core slot lists are built per (w,c);
    # shared count = max over cores (pad with cell 0 / colv -1).
    nslot_wc = np.zeros((NCORES, NWIN, NCHUNK), dtype=np.int64)
    slot_data = [[[None] * NCHUNK for _ in range(NWIN)] for _ in range(NCORES)]
    for k in range(NCORES):
        d = core[k]
        chunk = d["cell"] >> 15
        # group edges by (w, chunk, cell, half)
        keys = np.lexsort((d["half"], d["cell"], chunk, d["w"]))
        wS, cS = d["w"][keys], chunk[keys]
        cellS, halfS, tS = d["cell"][keys], d["half"][keys], d["t"][keys]
        # boundaries of (w, chunk) groups
        gkey = wS * NCHUNK + cS
        gb = np.flatnonzero(np.r_[True, gkey[1:] != gkey[:-1], True])
        for gi in range(len(gb) - 1):
            a, b = gb[gi], gb[gi + 1]
            wv, cv = int(wS[a]), int(cS[a])
            # within group: per cell, lo edges and hi edges
            cells_g = cellS[a:b]
            halves_g = halfS[a:b]
            ts_g = tS[a:b]
            cb = np.flatnonzero(np.r_[True, cells_g[1:] != cells_g[:-1], True])
            sc, slo, shi = [], [], []
            for ci in range(len(cb) - 1):
                p, q = cb[ci], cb[ci + 1]
                lo_t = ts_g[p:q][halves_g[p:q] == 0]
                hi_t = ts_g[p:q][halves_g[p:q] == 1]
                nsl = max(len(lo_t), len(hi_t))
                for j in range(nsl):
                    sc.append(cells_g[p])
                    slo.append(lo_t[j] if j < len(lo_t) else -1)
                    shi.append(hi_t[j] if j < len(hi_t) else -1)
            slot_data[k][wv][cv] = (np.asarray(sc, dtype=np.int64),
                                    np.asarray(slo, dtype=np.int64),
                                    np.asarray(shi, dtype=np.int64))
            nslot_wc[k, wv, cv] = len(sc)
    NSLOT_WC = nslot_wc.max(axis=0)               # shared [NWIN, NCHUNK]

    # ---- shared block/call/piece schedule -----------------------------
    # emission: for sw, for chunk: group slots = concat of windows' padded
    # segments; blocks of 128; calls of <= CALLCAP blocks; pieces per
    # (block, half, window-overlap).
    call_plan = []      # (s, c, cb)  per call
    piece_plan = []     # per call: list of (bi, half, m, start, stop)
    seg_off = np.zeros((NWIN, NCHUNK), dtype=np.int64)   # slot offset of (w,c)
    group_blocks = {}   # (s, c) -> nblocks
    TOTBLK = 0
    win_piece_count = np.zeros(NWIN, dtype=np.int64)
    # first pass: offsets and piece counts
    pieces_by_group = {}
    for s in range(NSW):
        ws = list(range(s * SW, min((s + 1) * SW, NWIN)))
        for c in range(NCHUNK):
            off = 0
            plist = []   # (bi, half, w, seg_a, seg_b) pieces
            for wv in ws:
                nwc = int(NSLOT_WC[wv, c])
                if wv == ws[0] and c == 0:
                    nwc = max(nwc, 1)   # every window gets >=1 piece overall
                seg_off[wv, c] = off
                if nwc == 0:
                    continue
                b0, b1_ = off >> 7, (off + nwc - 1) >> 7
                for bi in range(b0, b1_ + 1):
                    for half in range(2):
                        plist.append((bi, half, wv))
                off += nwc
            nb = (off + P - 1) >> 7 if off else 0
            group_blocks[(s, c)] = nb
            TOTBLK += nb
            pieces_by_group[(s, c)] = plist
    # piece flags (start/stop per window over global emission order)
    totals = np.zeros(NWIN, dtype=np.int64)
    for s in range(NSW):
        for c in range(NCHUNK):
            for (bi, half, wv) in pieces_by_group[(s, c)]:
                totals[wv] += 1
    seen = np.zeros(NWIN, dtype=np.int64)
    piece_flags = {}
    for s in range(NSW):
        for c in range(NCHUNK):
            fl = []
            for (bi, half, wv) in pieces_by_group[(s, c)]:
                seen[wv] += 1
                fl.append((seen[wv] == 1, seen[wv] == totals[wv]))
            piece_flags[(s, c)] = fl
    NPIECE = int(totals.sum())

    # ---- per-core slot/cell/colv arrays --------------------------------
    # slots laid out per (s, c) group: windows' segments at seg_off.
    TOTSLOT = TOTBLK * P
    slot_cell = np.zeros((NCORES, TOTSLOT), dtype=np.int64)
    colv = np.full((NCORES, P, NPIECE), -1.0, dtype=np.float32)
    # group slot base offsets in global slot space
    gbase = {}
    acc = 0
    for s in range(NSW):
        for c in range(NCHUNK):
            gbase[(s, c)] = acc
            acc += group_blocks[(s, c)] * P
    # piece column index (global), per group ordered
    pcol = {}
    acc = 0
    for s in range(NSW):
        for c in range(NCHUNK):
            pcol[(s, c)] = acc
            acc += len(pieces_by_group[(s, c)])
    for k in range(NCORES):
        for s in range(NSW):
            ws = list(range(s * SW, min((s + 1) * SW, NWIN)))
            for c in range(NCHUNK):
                base = gbase[(s, c)]
                for wv in ws:
                    sd = slot_data[k][wv][c]
                    if sd is None:
                        continue
                    sc, slo, shi = sd
                    nwc = len(sc)
                    if nwc == 0:
                        continue
                    o = base + int(seg_off[wv, c])
                    slot_cell[k, o:o + nwc] = sc - c * CHUNK_CELLS
                # colv per piece
                plist = pieces_by_group[(s, c)]
                for j, (bi, half, wv) in enumerate(plist):
                    sd = slot_data[k][wv][c]
                    if sd is None:
                        continue
                    sc, slo, shi = sd
                    nwc = len(sc)
                    if nwc == 0:
                        continue
                    o = int(seg_off[wv, c])
                    tv = slo if half == 0 else shi
                    # slots of this window inside block bi
                    lo_s = max(o, bi * P)
                    hi_s = min(o + nwc, (bi + 1) * P)
                    if lo_s >= hi_s:
                        continue
                    pidx = pcol[(s, c)] + j
                    pp = np.arange(lo_s, hi_s) & 127
                    colv[k, pp, pidx] = tv[lo_s - o:hi_s - o]

    # ---- calls + idx16 wrap -------------------------------------------
    # per (s, c): split blocks into calls of <= CALLCAP
    call_plan = []
    for s in range(NSW):
        for c in range(NCHUNK):
            nb = group_blocks[(s, c)]
            g0 = 0
            while g0 < nb:
                cb = min(CALLCAP, nb - g0)
                call_plan.append((s, c, g0, cb))
                g0 += cb
    idx16 = np.zeros((NCORES, P, TOTBLK * 8), dtype=np.int16)
    for k in range(NCORES):
        off16 = 0
        for (s, c, g0, cb) in call_plan:
            base = gbase[(s, c)] + g0 * P
            nslot = cb * P
            gidx = slot_cell[k, base:base + nslot].astype(np.int16)
            wr = gidx.reshape(-1, 16).T
            idx16[k, :, off16:off16 + nslot // 16] = np.tile(wr, (8, 1))
            off16 += nslot // 16

    # ---- per-core x tables --------------------------------------------
    NROWS = 2 * NCELL_CAP
    xtabs = np.zeros((NCORES, NROWS, FIN), dtype=bf16)
    for k in range(NCORES):
        cells = core[k]["cells"]
        nr = 2 * len(cells)
        rows = cells.reshape(-1)
        xtabs[k, :nr] = xs[rows]

    # ---- pooling / epilogue tensors (as v1) ----------------------------
    Bmat = np.zeros((NCORES, P, NWIN * GPC), dtype=bf16)
    dinv_win = np.zeros((NCORES, P, NWIN), dtype=np.float32)
    for k in range(NCORES):
        nn = int(nk[k])
        nodes = np.arange(starts[k], ends[k])
        g = batch[nodes] - k * GPC
        wv = np.arange(nn) >> 7
        pv = np.arange(nn) & 127
        Bm = np.zeros((P, NWIN, GPC), dtype=np.float32)
        Bm[pv, wv, g] = inv_cnt[batch[nodes]]
        Bmat[k] = Bm.reshape(P, NWIN * GPC).astype(bf16)
        dv = np.zeros((P, NWIN), dtype=np.float32)
        dv[pv, wv] = dinv[nodes]
        dinv_win[k] = dv

    b1b = np.broadcast_to(b1, (P, H)).astype(np.float32).copy()
    wfce = np.concatenate([Wfc, bfc[None, :]], axis=0).astype(np.float32)
    iota = np.broadcast_to(np.arange(P, dtype=np.float32), (P, P)).astype(bf16).copy()
    ident = np.eye(P, dtype=np.float32)

    return dict(
        NWIN=NWIN, NSW=NSW, TOTBLK=TOTBLK, NPIECE=NPIECE,
        call_plan=call_plan, pieces_by_group=pieces_by_group,
        piece_flags=piece_flags, group_blocks=group_blocks, pcol=pcol,
        idx16=idx16, colv=colv.astype(bf16), xtabs=xtabs, slot_cell=slot_cell,
        gbase=gbase, seg_off=seg_off,
        Bmat=Bmat, dinv_win=dinv_win, b1b=b1b, wfce=wfce, iota=iota,
        ident=ident, W1=W1.astype(bf16), NROWS=NROWS,
    )


# ------------------------------------------------------------------ kernel IR
def build_nc(sched, num_devices=NCORES):
    NWIN, NSW = sched["NWIN"], sched["NSW"]
    TOTBLK, NPIECE = sched["TOTBLK"], sched["NPIECE"]
    call_plan = sched["call_plan"]
    pieces_by_group = sched["pieces_by_group"]
    piece_flags = sched["piece_flags"]
    pcol = sched["pcol"]
    NROWS = sched["NROWS"]
    f32, bft, i16 = mybir.dt.float32, mybir.dt.bfloat16, mybir.dt.int16

    nc = bacc.Bacc("TRN2", target_bir_lowering=False, debug=False,
                   num_devices=num_devices)
    d_x = nc.dram_tensor("xtab", [NROWS, FIN], bft, kind="ExternalInput")
    d_W1 = nc.dram_tensor("W1", [FIN, H], bft, kind="ExternalInput")
    d_idx = nc.dram_tensor("idx16", [P, TOTBLK * 8], i16, kind="ExternalInput")
    d_colv = nc.dram_tensor("colv", [P, NPIECE], bft, kind="ExternalInput")
    d_B = nc.dram_tensor("Bmat", [P, NWIN * GPC], bft, kind="ExternalInput")
    d_dwin = nc.dram_tensor("dinv_win", [P, NWIN], f32, kind="ExternalInput")
    d_b1b = nc.dram_tensor("b1b", [P, H], f32, kind="ExternalInput")
    d_wfce = nc.dram_tensor("wfce", [H + 1, 2], f32, kind="ExternalInput")
    d_iota = nc.dram_tensor("iota", [P, P], bft, kind="ExternalInput")
    d_ident = nc.dram_tensor("ident", [P, P], f32, kind="ExternalInput")
    d_out = nc.dram_tensor("outd", [GPC, 2], f32, kind="ExternalOutput")

    cellsv = d_x.rearrange("(a b) f -> a (b f)", b=2)   # [NCELL_CAP*?, 256]

    NPC_MAX = max(len(pieces_by_group[g]) and 0 or 0 for g in pieces_by_group) \
        if False else 0
    # max pieces per call
    npc_call = []
    for (s, c, g0, cb) in call_plan:
        plist = pieces_by_group[(s, c)]
        npc = sum(1 for (bi, half, wv) in plist if g0 <= bi < g0 + cb)
        npc_call.append(npc)
    NPC_MAX = max(npc_call)

    with tile.TileContext(nc) as tc:
        with tc.tile_pool(name="const", bufs=1) as cp, \
             tc.tile_pool(name="gio", bufs=3) as gio, \
             tc.tile_pool(name="tp", bufs=3) as tpool, \
             tc.tile_pool(name="wio", bufs=3) as wio, \
             tc.tile_pool(name="hps", bufs=2, space="PSUM") as hps, \
             tc.tile_pool(name="aggps", bufs=SW, space="PSUM") as aggps, \
             tc.tile_pool(name="poolps", bufs=1, space="PSUM") as poolps:

            w1_t = cp.tile([FIN, H], bft, tag="w1")
            nc.sync.dma_start(out=w1_t[:], in_=d_W1[:])
            iota_t = cp.tile([P, P], bft, tag="iota")
            nc.sync.dma_start(out=iota_t[:], in_=d_iota[:])
            ident_t = cp.tile([P, P], f32, tag="ident")
            nc.sync.dma_start(out=ident_t[:], in_=d_ident[:])
            b1b_t = cp.tile([P, H], f32, tag="b1b")
            nc.sync.dma_start(out=b1b_t[:], in_=d_b1b[:])
            wfce_t = cp.tile([H + 1, 2], f32, tag="wfce")
            nc.sync.dma_start(out=wfce_t[:], in_=d_wfce[:])
            idx_t = cp.tile([P, TOTBLK * 8], i16, tag="idx")
            nc.sync.dma_start(out=idx_t[:], in_=d_idx[:])
            colv_t = cp.tile([P, NPIECE], bft, tag="colv")
            nc.sync.dma_start(out=colv_t[:], in_=d_colv[:])
            bmat_t = cp.tile([P, NWIN * GPC], bft, tag="bmat")
            nc.sync.dma_start(out=bmat_t[:], in_=d_B[:])
            dwin_t = cp.tile([P, NWIN], f32, tag="dwin")
            nc.sync.dma_start(out=dwin_t[:], in_=d_dwin[:])
            ones_t = cp.tile([P, 1], bft, tag="ones")
            nc.vector.memset(ones_t[:], 1.0)

            pool_ps = poolps.tile([H + 1, GPC], f32, tag="pool")
            ci = 0          # call index
            nwin_done = 0
            for s in range(NSW):
                ws = list(range(s * SW, min((s + 1) * SW, NWIN)))
                agg = {w: aggps.tile([P, P], f32, tag="agg", name=f"agg{w}")
                       for w in ws}
                for c in range(NCHUNK):
                    plist = pieces_by_group[(s, c)]
                    flist = piece_flags[(s, c)]
                    nb_group = sched["group_blocks"][(s, c)]
                    # calls for this group
                    off16_base = None
                    while ci < len(call_plan) and call_plan[ci][0] == s \
                            and call_plan[ci][1] == c:
                        _, _, g0, cb = call_plan[ci]
                        # idx16 offset: calls are consumed in plan order
                        off16 = sum(cp_[3] * 8 for cp_ in call_plan[:ci])
                        msg = gio.tile([P, CALLCAP, 2 * P], bft, tag="msg")
                        nc.gpsimd.dma_gather(
                            out_ap=msg[:, 0:cb, :],
                            in_ap=cellsv[c * CHUNK_CELLS:(c + 1) * CHUNK_CELLS, :],
                            idxs_ap=idx_t[:, off16:off16 + cb * 8],
                            num_idxs=cb * P, num_idxs_reg=cb * P,
                            elem_size=2 * P)
                        # pieces whose block is in this call
                        sel = [(j, bi, half, wv)
                               for j, (bi, half, wv) in enumerate(plist)
                               if g0 <= bi < g0 + cb]
                        if sel:
                            j0 = sel[0][0]
                            npc = len(sel)
                            pc0 = pcol[(s, c)] + j0
                            Tt = tpool.tile([P, NPC_MAX, P], bft, tag="T")
                            nc.vector.tensor_tensor(
                                out=Tt[:, 0:npc, :],
                                in0=iota_t[:, None, :].broadcast_to(
                                    [P, npc, P]),
                                in1=colv_t[:, pc0:pc0 + npc, None].broadcast_to(
                                    [P, npc, P]),
                                op=mybir.AluOpType.is_equal)
                            for jj, (j, bi, half, wv) in enumerate(sel):
                                first, last = flist[j]
                                nc.tensor.matmul(
                                    out=agg[wv][:],
                                    lhsT=msg[:, bi - g0,
                                             half * P:(half + 1) * P],
                                    rhs=Tt[:, jj, :],
                                    start=first, stop=last)
                        ci += 1
                # window epilogue
                for w in ws:
                    aggsb = wio.tile([P, P], bft, tag="aggsb")
                    nc.scalar.activation(
                        out=aggsb[:], in_=agg[w][:],
                        func=mybir.ActivationFunctionType.Copy)
                    h_ps = hps.tile([P, H], f32, tag="hp")
                    nc.tensor.matmul(out=h_ps[:], lhsT=aggsb[:], rhs=w1_t[:],
                                     start=True, stop=True)
                    sc = wio.tile([P, H], f32, tag="sc")
                    nc.vector.tensor_scalar(
                        out=sc[:], in0=h_ps[:], scalar1=dwin_t[:, w:w + 1],
                        scalar2=None, op0=mybir.AluOpType.mult)
                    sb = wio.tile([P, H], f32, tag="sb")
                    nc.vector.tensor_tensor(
                        out=sb[:], in0=sc[:], in1=b1b_t[:],
                        op=mybir.AluOpType.add)
                    rl = wio.tile([P, H], bft, tag="rl")
                    nc.scalar.activation(
                        out=rl[:], in_=sb[:],
                        func=mybir.ActivationFunctionType.Relu)
                    first = nwin_done == 0
                    last = nwin_done == NWIN - 1
                    nc.tensor.matmul(
                        out=pool_ps[0:H, :], lhsT=rl[:],
                        rhs=bmat_t[:, w * GPC:(w + 1) * GPC],
                        start=first, stop=last, skip_group_check=True)
                    nc.tensor.matmul(
                        out=pool_ps[H:H + 1, :], lhsT=ones_t[:],
                        rhs=bmat_t[:, w * GPC:(w + 1) * GPC],
                        start=first, stop=last, skip_group_check=True)
                    nwin_done += 1

            # ---- FC + log_softmax ----
            plc = cp.tile([H + 1, GPC], f32, tag="plc")
            nc.vector.tensor_copy(out=plc[:], in_=pool_ps[:])
            lg_ps = hps.tile([2, GPC], f32, tag="hp")
            nc.tensor.matmul(out=lg_ps[:], lhsT=wfce_t[:], rhs=plc[:],
                             start=True, stop=True)
            lgs = cp.tile([2, GPC], f32, tag="lgs")
            nc.vector.tensor_copy(out=lgs[:], in_=lg_ps[:])
            tr_ps = hps.tile([GPC, 2], f32, tag="hp")
            nc.tensor.transpose(out=tr_ps[:], in_=lgs[:], identity=ident_t[:2, :2])
            ls = cp.tile([GPC, 2], f32, tag="ls")
            nc.vector.tensor_copy(out=ls[:], in_=tr_ps[:])
            nm = cp.tile([GPC, 1], f32, tag="nm")
            nc.vector.tensor_reduce(out=nm[:], in_=ls[:],
                                    axis=mybir.AxisListType.X,
                                    op=mybir.AluOpType.max, negate=True)
            ex = cp.tile([GPC, 2], f32, tag="ex")
            nc.scalar.activation(out=ex[:], in_=ls[:],
                                 func=mybir.ActivationFunctionType.Exp,
                                 bias=nm[:, 0:1])
            ssum = cp.tile([GPC, 1], f32, tag="ssum")
            nc.vector.tensor_reduce(out=ssum[:], in_=ex[:],
                                    axis=mybir.AxisListType.X,
                                    op=mybir.AluOpType.add)
            lse = cp.tile([GPC, 1], f32, tag="lse")
            nc.scalar.activation(out=lse[:], in_=ssum[:],
                                 func=mybir.ActivationFunctionType.Ln)
            fin = cp.tile([GPC, 2], f32, tag="fin")
            nc.vector.tensor_scalar(
                out=fin[:], in0=ls[:], scalar1=nm[:, 0:1], scalar2=lse[:, 0:1],
                op0=mybir.AluOpType.add, op1=mybir.AluOpType.subtract)
            nc.sync.dma_start(out=d_out[:], in_=fin[:])

    nc.compile()
    return nc


def make_in_maps(sched):
    maps = []
    for k in range(NCORES):
        maps.append({
            "xtab": sched["xtabs"][k], "W1": sched["W1"],
            "idx16": sched["idx16"][k], "colv": sched["colv"][k],
            "Bmat": sched["Bmat"][k], "dinv_win": sched["dinv_win"][k],
            "b1b": sched["b1b"], "wfce": sched["wfce"],
            "iota": sched["iota"], "ident": sched["ident"],
        })
    return maps


def kernel(**inputs) -> np.ndarray:
    sched = build_schedule(**inputs)
    nc = build_nc(sched)
    res = bass_utils.run_bass_kernel_spmd(
        nc, make_in_maps(sched), core_ids=list(range(NCORES)))
    out = np.concatenate([res.results[k]["outd"] for k in range(NCORES)], axis=0)
    return out.astype(np.float32)


# revision 7
# speedup vs baseline: 3.2247x; 1.3833x over previous
"""CrystalGNN (GCNConv -> relu -> mean-pool -> FC -> log_softmax) on 8
Trainium2 NeuronCores — v3 (quad-cell gather).

Bottleneck: SWDGE descriptor generation on GpSimd costs ~7.7 ns per
dma_gather index regardless of element size. v1 gathered one 256B row
per edge (~290k idx/core -> 2.4 ms). v3 packs FOUR same-window edges'
source rows (256B bf16 each, pre-scaled by dinv[src]) into one 1024B
"cell", so one index feeds four edges (~57k idx/core). The per-core
cell table is built host-side (bounded duplication, 2 chunks of <=32768
cells for int16 indexing, chunk split balanced per window so the shared
SPMD slot layout pads minimally across cores).

Per gathered 128-slot block, per (quarter, window) "piece", a one-hot T
(built batched on DVE via stride-0 broadcast is_equal against iota)
routes one quarter of the message block into the window's PSUM
accumulator in x-space: aggT[f, t] += msg_q^T @ T. Each window then
applies W1 (tensor), dinv[tgt] + bias + relu (vector/scalar), and the
mean-pool matmul; a tiny FC + log_softmax tail finishes on-device.
Graph/data parallel across 8 cores per the sharding hint (batch_idx is
sorted, so each core owns 32 graphs and every edge targeting them).

Self-contained: only needs numpy/ml_dtypes + the concourse stack at
/opt/trn_rl_repo (or already on sys.path).
"""
import sys

for _p in ("/opt/trn_rl_repo",):
    if _p not in sys.path:
        sys.path.append(_p)

import numpy as np
import ml_dtypes

import concourse.bass as bass
import concourse.bacc as bacc
import concourse.mybir as mybir
import concourse.tile as tile
from concourse import bass_utils

P = 128
NCORES = 8
NGRAPH = 256
GPC = NGRAPH // NCORES        # graphs per core
N = 100000                    # nodes
FIN = 128                     # input features
H = 64                        # hidden
SW = 4                        # windows per superwindow (PSUM agg tiles)
CALLCAP = 8                   # max gather blocks (x128 idx) per dma_gather call
K = 4                         # edges (rows) per cell
CHUNK_CELLS = 32768           # cells addressable by int16 per chunk
NCHUNK = 2

bf16 = ml_dtypes.bfloat16


# ----------------------------------------------------------------- schedule
def build_schedule(x, edge_index, batch_idx, W1, b1, Wfc, bfc):
    x = np.asarray(x, dtype=np.float32)
    edge_index = np.asarray(edge_index).astype(np.int64)
    batch = np.asarray(batch_idx).astype(np.int64)
    W1 = np.asarray(W1, dtype=np.float32)
    b1 = np.asarray(b1, dtype=np.float32)
    Wfc = np.asarray(Wfc, dtype=np.float32)
    bfc = np.asarray(bfc, dtype=np.float32)

    row, col = edge_index[0], edge_index[1]
    deg = np.bincount(col, minlength=N).astype(np.float32) + 1.0
    dinv = (1.0 / np.sqrt(deg)).astype(np.float32)

    gcnt = np.bincount(batch, minlength=NGRAPH).astype(np.float32)
    assert (gcnt > 0).all(), "empty graphs unsupported"
    inv_cnt = 1.0 / gcnt

    starts = np.searchsorted(batch, np.arange(NCORES) * GPC, side="left")
    ends = np.searchsorted(batch, (np.arange(NCORES) + 1) * GPC, side="left")
    nk = ends - starts
    NWIN = int(np.ceil(nk.max() / P))
    NSW = (NWIN + SW - 1) // SW

    xs = (x * dinv[:, None]).astype(bf16)         # dinv[src]-scaled rows

    ecore = np.searchsorted(ends, col, side="right")

    # ---- per-core: group each window's edges into cells of K ----------
    # ncell_wc[k, w, c], and per (k, w, c): cell row-lists + target-lists
    ncell_wc = np.zeros((NCORES, NWIN, NCHUNK), dtype=np.int64)
    core_rows = []            # per core: [NCHUNK] lists of row arrays
    core_tgts = []            # per core: per (w, c): targets [ncell, K] (-1 pad)
    for k in range(NCORES):
        m = ecore == k
        nloc = np.arange(starts[k], ends[k], dtype=np.int64)
        esrc = np.concatenate([row[m], nloc])
        etl = np.concatenate([col[m] - starts[k], nloc - starts[k]])
        order = np.argsort(etl >> 7, kind="stable")
        esrc, etl = esrc[order], etl[order]
        ew, et = etl >> 7, etl & 127
        wb = np.searchsorted(ew, np.arange(NWIN + 1))
        rows_c = [[] for _ in range(NCHUNK)]
        tgts_wc = {}
        for wv in range(NWIN):
            a, b = int(wb[wv]), int(wb[wv + 1])
            ne = b - a
            nc_ = (ne + K - 1) // K
            srcs = np.concatenate([esrc[a:b],
                                   np.full(nc_ * K - ne, esrc[a] if ne else 0,
                                           dtype=np.int64)])
            tgts = np.concatenate([et[a:b],
                                   np.full(nc_ * K - ne, -1, dtype=np.int64)])
            srcs = srcs.reshape(nc_, K)
            tgts = tgts.reshape(nc_, K)
            h0 = (nc_ + 1) // 2                      # balanced chunk split
            for (s0, s1, c) in [(0, h0, 0), (h0, nc_, 1)]:
                ncell_wc[k, wv, c] = s1 - s0
                rows_c[c].append(srcs[s0:s1])
                tgts_wc[(wv, c)] = tgts[s0:s1]
        core_rows.append(rows_c)
        core_tgts.append(tgts_wc)

    NCELL_C = ncell_wc.sum(axis=1).max(axis=0)       # used cells per chunk
    assert (NCELL_C <= CHUNK_CELLS).all(), NCELL_C
    NSLOT_WC = ncell_wc.max(axis=0)                  # shared [NWIN, NCHUNK]

    # ---- shared block/call/piece schedule -----------------------------
    seg_off = np.zeros((NWIN, NCHUNK), dtype=np.int64)
    group_blocks = {}
    pieces_by_group = {}
    TOTBLK = 0
    for s in range(NSW):
        ws = list(range(s * SW, min((s + 1) * SW, NWIN)))
        for c in range(NCHUNK):
            off = 0
            plist = []
            for wv in ws:
                nwc = int(NSLOT_WC[wv, c])
                if wv == ws[0] and c == 0:
                    nwc = max(nwc, 1)
                seg_off[wv, c] = off
                if nwc == 0:
                    continue
                b0, b1_ = off >> 7, (off + nwc - 1) >> 7
                for bi in range(b0, b1_ + 1):
                    for q in range(K):
                        plist.append((bi, q, wv))
                off += nwc
            nb = (off + P - 1) >> 7 if off else 0
            group_blocks[(s, c)] = nb
            TOTBLK += nb
            pieces_by_group[(s, c)] = plist

    totals = np.zeros(NWIN, dtype=np.int64)
    for g, plist in pieces_by_group.items():
        for (bi, q, wv) in plist:
            totals[wv] += 1
    seen = np.zeros(NWIN, dtype=np.int64)
    piece_flags = {}
    for s in range(NSW):
        for c in range(NCHUNK):
            fl = []
            for (bi, q, wv) in pieces_by_group[(s, c)]:
                seen[wv] += 1
                fl.append((seen[wv] == 1, seen[wv] == totals[wv]))
            piece_flags[(s, c)] = fl
    NPIECE = int(totals.sum())

    gbase, pcol = {}, {}
    acc = accp = 0
    for s in range(NSW):
        for c in range(NCHUNK):
            gbase[(s, c)] = acc
            acc += group_blocks[(s, c)] * P
            pcol[(s, c)] = accp
            accp += len(pieces_by_group[(s, c)])
    TOTSLOT = TOTBLK * P

    # ---- per-core slot cells / colv / tables ---------------------------
    chunk_base = np.zeros(NCHUNK + 1, dtype=np.int64)
    chunk_base[1:] = np.cumsum(NCELL_C)
    NCELL_TOT = int(chunk_base[-1])
    NROWS = NCELL_TOT * K

    slot_cell = np.zeros((NCORES, TOTSLOT), dtype=np.int64)   # chunk-local id
    colv = np.full((NCORES, P, NPIECE), -1.0, dtype=np.float32)
    xtabs = np.zeros((NCORES, NROWS, FIN), dtype=bf16)
    for k in range(NCORES):
        # cells laid out per chunk: windows in order
        cur = [0] * NCHUNK
        cell_id_wc = {}
        for wv in range(NWIN):
            for c in range(NCHUNK):
                n = int(ncell_wc[k, wv, c])
                cell_id_wc[(wv, c)] = cur[c]
                cur[c] += n
        for c in range(NCHUNK):
            if core_rows[k][c]:
                rows = np.concatenate(core_rows[k][c]).reshape(-1)
                base = chunk_base[c] * K
                xtabs[k, base:base + len(rows)] = xs[rows]
        for s in range(NSW):
            ws = list(range(s * SW, min((s + 1) * SW, NWIN)))
            for c in range(NCHUNK):
                base = gbase[(s, c)]
                for wv in ws:
                    n = int(ncell_wc[k, wv, c])
                    if n == 0:
                        continue
                    o = base + int(seg_off[wv, c])
                    slot_cell[k, o:o + n] = cell_id_wc[(wv, c)] + np.arange(n)
                plist = pieces_by_group[(s, c)]
                for j, (bi, q, wv) in enumerate(plist):
                    n = int(ncell_wc[k, wv, c])
                    if n == 0:
                        continue
                    o = int(seg_off[wv, c])
                    lo_s = max(o, bi * P)
                    hi_s = min(o + n, (bi + 1) * P)
                    if lo_s >= hi_s:
                        continue
                    tq = core_tgts[k][(wv, c)][:, q]
                    pidx = pcol[(s, c)] + j
                    pp = np.arange(lo_s, hi_s) & 127
                    colv[k, pp, pidx] = tq[lo_s - o:hi_s - o]

    # ---- calls + idx16 wrap -------------------------------------------
    call_plan = []
    for s in range(NSW):
        for c in range(NCHUNK):
            nb = group_blocks[(s, c)]
            g0 = 0
            while g0 < nb:
                cb = min(CALLCAP, nb - g0)
                call_plan.append((s, c, g0, cb))
                g0 += cb
    idx16 = np.zeros((NCORES, P, TOTBLK * 8), dtype=np.int16)
    for k in range(NCORES):
        off16 = 0
        for (s, c, g0, cb) in call_plan:
            base = gbase[(s, c)] + g0 * P
            nslot = cb * P
            gidx = slot_cell[k, base:base + nslot].astype(np.int16)
            wr = gidx.reshape(-1, 16).T
            idx16[k, :, off16:off16 + nslot // 16] = np.tile(wr, (8, 1))
            off16 += nslot // 16

    # ---- pooling / epilogue tensors ------------------------------------
    Bmat = np.zeros((NCORES, P, NWIN * GPC), dtype=bf16)
    dinv_win = np.zeros((NCORES, P, NWIN), dtype=np.float32)
    for k in range(NCORES):
        nn = int(nk[k])
        nodes = np.arange(starts[k], ends[k])
        g = batch[nodes] - k * GPC
        wv = np.arange(nn) >> 7
        pv = np.arange(nn) & 127
        Bm = np.zeros((P, NWIN, GPC), dtype=np.float32)
        Bm[pv, wv, g] = inv_cnt[batch[nodes]]
        Bmat[k] = Bm.reshape(P, NWIN * GPC).astype(bf16)
        dv = np.zeros((P, NWIN), dtype=np.float32)
        dv[pv, wv] = dinv[nodes]
        dinv_win[k] = dv

    b1b = np.broadcast_to(b1, (P, H)).astype(np.float32).copy()
    wfce = np.concatenate([Wfc, bfc[None, :]], axis=0).astype(np.float32)
    iota = np.broadcast_to(np.arange(P, dtype=np.float32), (P, P)).astype(bf16).copy()
    ident = np.eye(P, dtype=np.float32)

    return dict(
        NWIN=NWIN, NSW=NSW, TOTBLK=TOTBLK, NPIECE=NPIECE, NROWS=NROWS,
        call_plan=call_plan, pieces_by_group=pieces_by_group,
        piece_flags=piece_flags, group_blocks=group_blocks, pcol=pcol,
        gbase=gbase, seg_off=seg_off, chunk_base=chunk_base,
        idx16=idx16, colv=colv.astype(bf16), xtabs=xtabs, slot_cell=slot_cell,
        Bmat=Bmat, dinv_win=dinv_win, b1b=b1b, wfce=wfce, iota=iota,
        ident=ident, W1=W1.astype(bf16),
    )


# ------------------------------------------------------------------ kernel IR
def build_nc(sched, num_devices=NCORES):
    NWIN, NSW = sched["NWIN"], sched["NSW"]
    TOTBLK, NPIECE = sched["TOTBLK"], sched["NPIECE"]
    call_plan = sched["call_plan"]
    pieces_by_group = sched["pieces_by_group"]
    piece_flags = sched["piece_flags"]
    pcol = sched["pcol"]
    chunk_base = sched["chunk_base"]
    NROWS = sched["NROWS"]
    f32, bft, i16 = mybir.dt.float32, mybir.dt.bfloat16, mybir.dt.int16

    nc = bacc.Bacc("TRN2", target_bir_lowering=False, debug=False,
                   num_devices=num_devices)
    d_x = nc.dram_tensor("xtab", [NROWS, FIN], bft, kind="ExternalInput")
    d_W1 = nc.dram_tensor("W1", [FIN, H], bft, kind="ExternalInput")
    d_idx = nc.dram_tensor("idx16", [P, TOTBLK * 8], i16, kind="ExternalInput")
    d_colv = nc.dram_tensor("colv", [P, NPIECE], bft, kind="ExternalInput")
    d_B = nc.dram_tensor("Bmat", [P, NWIN * GPC], bft, kind="ExternalInput")
    d_dwin = nc.dram_tensor("dinv_win", [P, NWIN], f32, kind="ExternalInput")
    d_b1b = nc.dram_tensor("b1b", [P, H], f32, kind="ExternalInput")
    d_wfce = nc.dram_tensor("wfce", [H + 1, 2], f32, kind="ExternalInput")
    d_iota = nc.dram_tensor("iota", [P, P], bft, kind="ExternalInput")
    d_ident = nc.dram_tensor("ident", [P, P], f32, kind="ExternalInput")
    d_out = nc.dram_tensor("outd", [GPC, 2], f32, kind="ExternalOutput")

    cellsv = d_x.rearrange("(a b) f -> a (b f)", b=K)   # [NCELL_TOT, K*128]

    npc_call = []
    for (s, c, g0, cb) in call_plan:
        plist = pieces_by_group[(s, c)]
        npc_call.append(sum(1 for (bi, q, wv) in plist if g0 <= bi < g0 + cb))
    NPC_MAX = max(npc_call)

    with tile.TileContext(nc) as tc:
        with tc.tile_pool(name="const", bufs=1) as cp, \
             tc.tile_pool(name="gio", bufs=3) as gio, \
             tc.tile_pool(name="tp", bufs=3) as tpool, \
             tc.tile_pool(name="wio", bufs=3) as wio, \
             tc.tile_pool(name="hps", bufs=2, space="PSUM") as hps, \
             tc.tile_pool(name="aggps", bufs=SW, space="PSUM") as aggps, \
             tc.tile_pool(name="poolps", bufs=1, space="PSUM") as poolps:

            w1_t = cp.tile([FIN, H], bft, tag="w1")
            nc.sync.dma_start(out=w1_t[:], in_=d_W1[:])
            iota_t = cp.tile([P, P], bft, tag="iota")
            nc.sync.dma_start(out=iota_t[:], in_=d_iota[:])
            ident_t = cp.tile([P, P], f32, tag="ident")
            nc.sync.dma_start(out=ident_t[:], in_=d_ident[:])
            b1b_t = cp.tile([P, H], f32, tag="b1b")
            nc.sync.dma_start(out=b1b_t[:], in_=d_b1b[:])
            wfce_t = cp.tile([H + 1, 2], f32, tag="wfce")
            nc.sync.dma_start(out=wfce_t[:], in_=d_wfce[:])
            idx_t = cp.tile([P, TOTBLK * 8], i16, tag="idx")
            nc.sync.dma_start(out=idx_t[:], in_=d_idx[:])
            colv_t = cp.tile([P, NPIECE], bft, tag="colv")
            nc.sync.dma_start(out=colv_t[:], in_=d_colv[:])
            bmat_t = cp.tile([P, NWIN * GPC], bft, tag="bmat")
            nc.sync.dma_start(out=bmat_t[:], in_=d_B[:])
            dwin_t = cp.tile([P, NWIN], f32, tag="dwin")
            nc.sync.dma_start(out=dwin_t[:], in_=d_dwin[:])
            ones_t = cp.tile([P, 1], bft, tag="ones")
            nc.vector.memset(ones_t[:], 1.0)

            pool_ps = poolps.tile([H + 1, GPC], f32, tag="pool")
            ci = 0
            off16 = 0
            nwin_done = 0
            for s in range(NSW):
                ws = list(range(s * SW, min((s + 1) * SW, NWIN)))
                agg = {w: aggps.tile([P, P], f32, tag="agg", name=f"agg{w}")
                       for w in ws}
                for c in range(NCHUNK):
                    plist = pieces_by_group[(s, c)]
                    flist = piece_flags[(s, c)]
                    cb0 = int(chunk_base[c])
                    cb1 = int(chunk_base[c + 1])
                    while ci < len(call_plan) and call_plan[ci][0] == s \
                            and call_plan[ci][1] == c:
                        _, _, g0, cb = call_plan[ci]
                        msg = gio.tile([P, CALLCAP, K * P], bft, tag="msg")
                        nc.gpsimd.dma_gather(
                            out_ap=msg[:, 0:cb, :],
                            in_ap=cellsv[cb0:cb1, :],
                            idxs_ap=idx_t[:, off16:off16 + cb * 8],
                            num_idxs=cb * P, num_idxs_reg=cb * P,
                            elem_size=K * P)
                        off16 += cb * 8
                        sel = [(j, bi, q, wv)
                               for j, (bi, q, wv) in enumerate(plist)
                               if g0 <= bi < g0 + cb]
                        if sel:
                            j0 = sel[0][0]
                            npc = len(sel)
                            assert [x[0] for x in sel] == \
                                list(range(j0, j0 + npc))
                            pc0 = pcol[(s, c)] + j0
                            Tt = tpool.tile([P, NPC_MAX, P], bft, tag="T")
                            nc.vector.tensor_tensor(
                                out=Tt[:, 0:npc, :],
                                in0=iota_t[:, None, :].broadcast_to(
                                    [P, npc, P]),
                                in1=colv_t[:, pc0:pc0 + npc, None].broadcast_to(
                                    [P, npc, P]),
                                op=mybir.AluOpType.is_equal)
                            for jj, (j, bi, q, wv) in enumerate(sel):
                                first, last = flist[j]
                                nc.tensor.matmul(
                                    out=agg[wv][:],
                                    lhsT=msg[:, bi - g0,
                                             q * P:(q + 1) * P],
                                    rhs=Tt[:, jj, :],
                                    start=first, stop=last)
                        ci += 1
                for w in ws:
                    aggsb = wio.tile([P, P], bft, tag="aggsb")
                    nc.scalar.activation(
                        out=aggsb[:], in_=agg[w][:],
                        func=mybir.ActivationFunctionType.Copy)
                    h_ps = hps.tile([P, H], f32, tag="hp")
                    nc.tensor.matmul(out=h_ps[:], lhsT=aggsb[:], rhs=w1_t[:],
                                     start=True, stop=True)
                    sc = wio.tile([P, H], f32, tag="sc")
                    nc.vector.tensor_scalar(
                        out=sc[:], in0=h_ps[:], scalar1=dwin_t[:, w:w + 1],
                        scalar2=None, op0=mybir.AluOpType.mult)
                    sb = wio.tile([P, H], f32, tag="sb")
                    nc.vector.tensor_tensor(
                        out=sb[:], in0=sc[:], in1=b1b_t[:],
                        op=mybir.AluOpType.add)
                    rl = wio.tile([P, H], bft, tag="rl")
                    nc.scalar.activation(
                        out=rl[:], in_=sb[:],
                        func=mybir.ActivationFunctionType.Relu)
                    first = nwin_done == 0
                    last = nwin_done == NWIN - 1
                    nc.tensor.matmul(
                        out=pool_ps[0:H, :], lhsT=rl[:],
                        rhs=bmat_t[:, w * GPC:(w + 1) * GPC],
                        start=first, stop=last, skip_group_check=True)
                    nc.tensor.matmul(
                        out=pool_ps[H:H + 1, :], lhsT=ones_t[:],
                        rhs=bmat_t[:, w * GPC:(w + 1) * GPC],
                        start=first, stop=last, skip_group_check=True)
                    nwin_done += 1

            # ---- FC + log_softmax ----
            plc = cp.tile([H + 1, GPC], f32, tag="plc")
            nc.vector.tensor_copy(out=plc[:], in_=pool_ps[:])
            lg_ps = hps.tile([2, GPC], f32, tag="hp")
            nc.tensor.matmul(out=lg_ps[:], lhsT=wfce_t[:], rhs=plc[:],
                             start=True, stop=True)
            lgs = cp.tile([2, GPC], f32, tag="lgs")
            nc.vector.tensor_copy(out=lgs[:], in_=lg_ps[:])
            tr_ps = hps.tile([GPC, 2], f32, tag="hp")
            nc.tensor.transpose(out=tr_ps[:], in_=lgs[:], identity=ident_t[:2, :2])
            ls = cp.tile([GPC, 2], f32, tag="ls")
            nc.vector.tensor_copy(out=ls[:], in_=tr_ps[:])
            nm = cp.tile([GPC, 1], f32, tag="nm")
            nc.vector.tensor_reduce(out=nm[:], in_=ls[:],
                                    axis=mybir.AxisListType.X,
                                    op=mybir.AluOpType.max, negate=True)
            ex = cp.tile([GPC, 2], f32, tag="ex")
            nc.scalar.activation(out=ex[:], in_=ls[:],
                                 func=mybir.ActivationFunctionType.Exp,
                                 bias=nm[:, 0:1])
            ssum = cp.tile([GPC, 1], f32, tag="ssum")
            nc.vector.tensor_reduce(out=ssum[:], in_=ex[:],
                                    axis=mybir.AxisListType.X,
                                    op=mybir.AluOpType.add)
            lse = cp.tile([GPC, 1], f32, tag="lse")
            nc.scalar.activation(out=lse[:], in_=ssum[:],
                                 func=mybir.ActivationFunctionType.Ln)
            fin = cp.tile([GPC, 2], f32, tag="fin")
            nc.vector.tensor_scalar(
                out=fin[:], in0=ls[:], scalar1=nm[:, 0:1], scalar2=lse[:, 0:1],
                op0=mybir.AluOpType.add, op1=mybir.AluOpType.subtract)
            nc.sync.dma_start(out=d_out[:], in_=fin[:])

    nc.compile()
    return nc


def make_in_maps(sched):
    maps = []
    for k in range(NCORES):
        maps.append({
            "xtab": sched["xtabs"][k], "W1": sched["W1"],
            "idx16": sched["idx16"][k], "colv": sched["colv"][k],
            "Bmat": sched["Bmat"][k], "dinv_win": sched["dinv_win"][k],
            "b1b": sched["b1b"], "wfce": sched["wfce"],
            "iota": sched["iota"], "ident": sched["ident"],
        })
    return maps


def kernel(**inputs) -> np.ndarray:
    sched = build_schedule(**inputs)
    nc = build_nc(sched)
    res = bass_utils.run_bass_kernel_spmd(
        nc, make_in_maps(sched), core_ids=list(range(NCORES)))
    out = np.concatenate([res.results[k]["outd"] for k in range(NCORES)], axis=0)
    return out.astype(np.float32)
